# revision 37
# baseline (speedup 1.0000x reference)
"""Trainium2 Bass kernel for CategoricalEntropyRegLoss.

Math: both loss terms factor so the [B,B] pairwise matrices are never built.

  feat_dists = sq_j + sq_k - 2 fn_j.fn_k            (rank FD+2)
  target_dists = (E_j - P_j.LQ_k) / D               (rank DC+1)
  S = sum_{jk} m_j m_k feat_dists * target_dists    (diag is exactly 0)
    = [ se*M + a*e - 2 Fe.F - Psq.L - Pbar.Lsq + 2 <U,V> ] / D
  tightness*M = a - sum_s ||seg_sum_s||^2 / max(cnt_s,1)

Everything needed is one matmul per core:
  out[1154, 258] = ext_seg^T @ ext_feat
  ext_seg  = [ onehot(code) | LQ' | P | 1 | E' ]    (B x 1154)
  ext_feat = [ m*fn | m | m ]                       (B x 258)
followed by a single 8-core AllReduce of the [128,10,258] partials and
a cheap redundant epilogue on every core.

Precision: all matmul operands and the AllReduce payload are fp16.
Raw log-probs (~ -3.5) would make the E/LQ-derived scalars (e, se ~
-1.7e3 per core) lose ~8 ulps per fp16 ring hop, which the ~7x
cancellation in the total amplifies past the threshold. The pairwise
KL is invariant under LQ += a, E += D*a, so we use LQ' = Ln(32*p)
(adds ln 32) and compute E' from LQ' -- this centers every stats value
near zero, the epilogue is unchanged, and end-to-end rel err lands at
~2.4e-4 measured vs the 2e-2 gate.

Scheduling notes (from perfetto traces):
 - fp32 matmul runs at 1/4 PE rate; fp16 operands cut the matmul phase
   ~20us -> ~5us. PSUM accumulates fp32 either way.
 - the collective costs ~11us fixed setup after its trigger plus the
   mesh phases (~19us for the 660KB fp16 payload); splitting into two
   collectives serializes on the one CC core and pays the fixed costs
   twice -- a single fused payload wins.
 - argmax is invariant under the positive per-row prob normalization,
   so the one-hot chain runs on raw targets the moment they land.
 - minv = m/||x|| comes from one ACT Abs_reciprocal_sqrt op (all-ones
   mask fast path; measured no accuracy impact) so the ef scale never
   waits on the DVE queue. A general masked variant (Sqrt + DVE
   reciprocal) is compiled on demand when mask has zeros.
 - inputs load with row-interleaved layout (partition p holds rows
   4p..4p+3): every reduced quantity is row-permutation invariant and
   descriptor lines go from 1KB/256B to 4KB/1KB contiguous.
 - cross-core launch skew (0-20us, run-to-run) is absorbed at the
   collective rendezvous; it is harness noise, not kernel time.
"""

import numpy as np

B = 4096
FD = 256
C = 32
D = 2
NSEG = C ** D          # 1024
NCORES = 8
RB = B // NCORES       # 512 rows per core
KT = RB // 128         # 4 k-chunks of 128 rows
EF = FD + 2            # 258: [mfn | m | m]
NST = 2 * D * C + 2    # 130 stats columns: [lq | p | ones | E]
NSMT = NSEG // 128     # 8 seg m-tiles
NMT = NSMT + 2         # 10 payload m-tiles

_compiled = {}


def _build_bass(all_ones):
    from contextlib import ExitStack
    import concourse.bass as bass
    import concourse.bacc as bacc
    import concourse.tile as tile
    from concourse import mybir

    from concourse.tile import add_dep_helper

    f32 = mybir.dt.float32
    f16 = mybir.dt.float16
    Alu = mybir.AluOpType
    Act = mybir.ActivationFunctionType
    Ax = mybir.AxisListType

    nc = bacc.Bacc(num_devices=NCORES)

    feat = nc.dram_tensor("features", [RB, FD], f32, kind="ExternalInput")
    targ = nc.dram_tensor("targets", [RB, D * C], f32, kind="ExternalInput")
    maskf = None
    if not all_ones:
        maskf = nc.dram_tensor("maskf", [RB, 1], f32, kind="ExternalInput")
    outd = nc.dram_tensor("out", [8], f32, kind="ExternalOutput")

    with ExitStack() as ctx:
        tc = ctx.enter_context(tile.TileContext(nc))
        consts = ctx.enter_context(tc.tile_pool(name="consts", bufs=1))
        work = ctx.enter_context(tc.tile_pool(name="work", bufs=1))
        keep = ctx.enter_context(tc.tile_pool(name="keep", bufs=1))
        psum = ctx.enter_context(tc.tile_pool(name="psum", bufs=1, space="PSUM"))
        dram = ctx.enter_context(tc.tile_pool(name="dram", bufs=1, space="DRAM"))

        # ---------------- constants ----------------
        ones128 = consts.tile([128, 1], f32)
        nc.vector.memset(ones128[:], 1.0)

        # partition-major fp16 payload [p, mt, f]: mt 0-7 seg, 8-9 stats
        inb = dram.tile([128, NMT, EF], f16, name="inb")
        outb = dram.tile([128, NMT, EF], f16, name="outb", addr_space="Shared")

        # ---- input loads spread over the queues (chunk-wise for early
        # arrival; squares are input-gated) ----
        # row-interleaved layout: partition p holds rows 4p..4p+3, chunk a =
        # a-th row of each partition. Every reduced quantity is permutation
        # invariant over rows, and this layout gives 4KB/1KB contiguous
        # descriptor lines per partition instead of 1KB/256B.
        tbig = keep.tile([128, KT, D * C], f32, name="tbig")
        nc.scalar.dma_start(
            out=tbig[:], in_=targ[:, :].rearrange("(p a) f -> p a f", p=128))
        mkbig = None
        if not all_ones:
            mkbig = keep.tile([128, KT, 1], f32, name="mkbig")
            nc.scalar.dma_start(
                out=mkbig[:],
                in_=maskf[:, :].rearrange("(p a) f -> p a f", p=128))
        xbig = keep.tile([128, KT, FD], f32, name="xbig")
        xflat = xbig[:, :, :].rearrange("p a f -> p (a f)")
        nc.sync.dma_start(
            out=xflat[0:44, :],
            in_=feat[0:176, :].rearrange("(p a) f -> p (a f)", p=44))
        nc.gpsimd.dma_start(
            out=xflat[44:88, :],
            in_=feat[176:352, :].rearrange("(p a) f -> p (a f)", p=44))
        nc.scalar.dma_start(
            out=xflat[88:128, :],
            in_=feat[352:512, :].rearrange("(p a) f -> p (a f)", p=40))

        def xchunk(kc):
            return xbig[:, kc, :]

        # iota after the gpsimd input DMA triggers (fp16: ints exact to 2048)
        iota1024 = consts.tile([128, NSEG], f16)
        nc.gpsimd.iota(iota1024[:], [[1, NSEG]], channel_multiplier=0,
                       allow_small_or_imprecise_dtypes=True)
        # biota[j] = 32 - j  (for first-argmax via reduce_max)
        biota = consts.tile([128, C], f32)
        nc.gpsimd.iota(biota[:], [[-1, C]], base=C, channel_multiplier=0,
                       allow_small_or_imprecise_dtypes=True)

        es_oh = [keep.tile([128, NSEG], f16, name=f"esoh_{kc}")
                 for kc in range(KT)]
        es_st = [keep.tile([128, NST], f32, name=f"esst_{kc}")
                 for kc in range(KT)]
        es_stb = [keep.tile([128, NST], f16, name=f"esstb_{kc}")
                  for kc in range(KT)]
        ef_b = [keep.tile([128, EF], f16, name=f"efb_{kc}")
                for kc in range(KT)]
        for kc in range(KT):
            nc.vector.memset(es_st[kc][:, NST - 2:NST - 1], 1.0)

        # ---- ACT phase 1: row sum-of-squares (Square table loads once) ----
        sqpack = keep.tile([128, KT], f32, name="sqpack")
        scrsq = keep.tile([128, FD], f32, name="scrsq")
        act_chain = []
        for kc in range(KT):
            act_chain.append(nc.scalar.activation(
                out=scrsq[:], in_=xchunk(kc), func=Act.Square,
                accum_out=sqpack[:, kc:kc + 1]))

        # ---- targets chains (DVE) ----
        # es_st columns: [0:64 lq' | 64:128 p | 128 ones | 129 E']
        t1big = keep.tile([128, KT, D * C], f32, name="t1big")
        nc.vector.tensor_scalar_add(out=t1big[:], in0=tbig[:], scalar1=1e-10)
        invsb = keep.tile([128, KT * D], f32, name="invsb")
        nc.vector.reduce_sum(
            out=invsb[:],
            in_=t1big[:].rearrange("p a (d c) -> p (a d) c", c=C),
            axis=Ax.X)
        nc.vector.reciprocal(invsb[:], invsb[:])
        # per-chunk first-argmax + one-hot on RAW targets (argmax is
        # invariant under the positive per-row normalization), DVE only —
        # runs the moment targets land, independent of the norm chain
        def argmax_onehot(kc):
            cls = work.tile([128, D], f32, name=f"cls_{kc}", tag=f"cl_{kc}")
            for d_ in range(D):
                pch = t1big[:, kc, C * d_:C * (d_ + 1)]
                mx = work.tile([128, 1], f32, name=f"mx_{kc}_{d_}",
                               tag=f"mx_{kc}_{d_}")
                nc.vector.reduce_max(out=mx[:], in_=pch, axis=Ax.X)
                cand = work.tile([128, C], f32, name=f"cand_{kc}_{d_}",
                                 tag=f"cd_{kc}_{d_}")
                # (p == max) * (32 - idx); reduce_max -> 32 - first_argmax
                nc.vector.scalar_tensor_tensor(
                    out=cand[:], in0=pch, scalar=mx[:], in1=biota[:],
                    op0=Alu.is_equal, op1=Alu.mult)
                mq = work.tile([128, 1], f32, name=f"mq_{kc}_{d_}",
                               tag=f"mq_{kc}_{d_}")
                nc.vector.reduce_max(out=mq[:], in_=cand[:], axis=Ax.X)
                nc.vector.tensor_scalar(
                    out=cls[:, d_:d_ + 1], in0=mq[:], scalar1=-1.0,
                    scalar2=float(C), op0=Alu.mult, op1=Alu.add)
            code = work.tile([128, 1], f32, name=f"code_{kc}", tag=f"co_{kc}")
            nc.vector.tensor_scalar(
                out=code[:], in0=cls[:, 1:2], scalar1=float(C),
                scalar2=cls[:, 0:1], op0=Alu.mult, op1=Alu.add)
            # one-hot straight to fp16 (ints <= 1023 exact)
            nc.vector.tensor_scalar(
                out=es_oh[kc][:], in0=iota1024[:], scalar1=code[:],
                scalar2=None, op0=Alu.is_equal)

        for kc in range(KT):
            argmax_onehot(kc)

        # ---- minv = m/norm, kept off the DVE critical queue where possible
        minvpack = keep.tile([128, KT], f32, name="minvpack")
        if all_ones:
            # single ACT op right after the squares
            act_chain.append(nc.scalar.activation(
                out=minvpack[:], in_=sqpack[:],
                func=Act.Abs_reciprocal_sqrt))
        else:
            normpack = keep.tile([128, KT], f32, name="normpack")
            act_chain.append(nc.scalar.sqrt(normpack[:], sqpack[:]))
            nc.vector.tensor_scalar_max(out=normpack[:], in0=normpack[:],
                                        scalar1=1e-12)
            invpack = keep.tile([128, KT], f32, name="invpack")
            nc.vector.reciprocal(invpack[:], normpack[:])
            nc.vector.tensor_tensor(out=minvpack[:], in0=invpack[:],
                                    in1=mkbig[:, :, 0], op=Alu.mult)

        # ---- ext_feat = [x*(m*inv) | m | m] in fp16 (ACT phase 3,
        # Copy is table-less so ef is ready before the Lns) ----
        for kc in range(KT):
            ef_t = ef_b[kc]
            act_chain.append(nc.scalar.activation(
                out=ef_t[:, 0:FD], in_=xchunk(kc), func=Act.Copy,
                scale=minvpack[:, kc:kc + 1]))
            if all_ones:
                nc.vector.memset(ef_t[:, FD:FD + 2], 1.0)
            else:
                nc.vector.tensor_copy(out=ef_t[:, FD:FD + 1],
                                      in_=mkbig[:, kc, :])
                nc.vector.tensor_copy(out=ef_t[:, FD + 1:FD + 2],
                                      in_=mkbig[:, kc, :])

        # ---- normalized probs (needed only by the stats operands) ----
        for kc in range(KT):
            pt = es_st[kc][:, D * C:2 * D * C]
            for d_ in range(D):
                nc.vector.tensor_scalar_mul(
                    out=pt[:, C * d_:C * (d_ + 1)],
                    in0=t1big[:, kc, C * d_:C * (d_ + 1)],
                    scalar1=invsb[:, kc * D + d_:kc * D + d_ + 1])

        # ---- ACT phase 4: Lns (stats operands are needed last) ----
        # lq' = Ln(32*p) = Ln(p) + ln 32: centers the log-probs so the
        # fp16 payload ring stays well conditioned (KL is invariant)
        for kc in range(KT):
            act_chain.append(nc.scalar.activation(
                out=es_st[kc][:, 0:D * C], in_=es_st[kc][:, D * C:2 * D * C],
                func=Act.Ln, scale=32.0))

        # E' / ones columns + fp16 operand copy of the stats block
        for kc in range(KT):
            st_t = es_st[kc]
            scr64 = work.tile([128, D * C], f32, name=f"scr64_{kc}",
                              tag=f"s64_{kc}")
            nc.vector.tensor_tensor(out=scr64[:],
                                    in0=st_t[:, D * C:2 * D * C],
                                    in1=st_t[:, 0:D * C], op=Alu.mult)
            nc.vector.reduce_sum(out=st_t[:, NST - 1:NST], in_=scr64[:],
                                 axis=Ax.X)
            nc.vector.tensor_copy(out=es_stb[kc][:], in_=st_t[:])

        # keep ACT ops grouped by function (avoid act-table reload thrash):
        # Square x4 -> Ln x4 -> Sqrt -> Copy x4 (table-less)
        for a, b in zip(act_chain[1:], act_chain[:-1]):
            add_dep_helper(a.ins, b.ins, sync=False,
                           reason="act table grouping")

        # ---------------- the one big matmul ----------------
        resa = keep.tile([128, 4, EF], f16, name="resa")
        resb = keep.tile([128, 4, EF], f16, name="resb")
        resc = keep.tile([128, 2, EF], f16, name="resc")
        nc.vector.memset(resc[:], 0.0)
        for mt in range(NMT):
            mlo = mt * 128
            msz = min(128, 1154 - mlo)
            ps = psum.tile([msz, EF], f32, name=f"ps_{mt}", tag=f"ps_{mt % 7}")
            for kc in range(KT):
                if mt < NSMT:
                    lhsT = es_oh[kc][:, mlo:mlo + msz]
                else:
                    lhsT = es_stb[kc][:, mlo - NSEG:mlo - NSEG + msz]
                nc.tensor.matmul(out=ps[:], lhsT=lhsT, rhs=ef_b[kc][:],
                                 start=(kc == 0), stop=(kc == KT - 1))
            if mt < 4:
                nc.vector.tensor_copy(out=resa[:, mt, :], in_=ps[:])
            elif mt < 8:
                nc.vector.tensor_copy(out=resb[:, mt - 4, :], in_=ps[:])
            else:
                nc.vector.tensor_copy(out=resc[0:msz, mt - 8, :], in_=ps[:])
            if mt == 3:
                nc.sync.dma_start(out=inb[:, 0:4, :], in_=resa[:])
            elif mt == 7:
                nc.gpsimd.dma_start(out=inb[:, 4:8, :], in_=resb[:])
            elif mt == 9:
                nc.scalar.dma_start(out=inb[:, 8:10, :], in_=resc[:])

        # ---------------- single AllReduce (fp16) ----------------
        nc.gpsimd.collective_compute(
            "AllReduce", mybir.AluOpType.add,
            replica_groups=[list(range(NCORES))],
            ins=[inb.opt()], outs=[outb.opt()])

        # ---------------- epilogue (redundant on every core) ----------------
        big0 = keep.tile([128, 4, EF], f16, name="big0")
        nc.sync.dma_start(out=big0[:], in_=outb[:, 0:4, :])
        big1 = keep.tile([128, 4, EF], f16, name="big1")
        nc.gpsimd.dma_start(out=big1[:], in_=outb[:, 4:8, :])
        # stats rows: partition-shifted copies so DVE lanes align
        ut = keep.tile([64, EF], f16, name="ut")
        nc.scalar.dma_start(out=ut[:], in_=outb[0:64, 8, :])
        vt = keep.tile([64, EF], f16, name="vt")
        nc.scalar.dma_start(out=vt[:], in_=outb[64:128, 8, :])
        last2 = keep.tile([1, EF], f16, name="last2")
        nc.scalar.dma_start(out=last2[:], in_=outb[0:1, 9, :])
        r1 = keep.tile([1, EF], f16, name="r1")
        nc.sync.dma_start(out=r1[:], in_=outb[1:2, 9, :])

        Z = keep.tile([128, 8], f32, name="Z")
        nc.vector.memset(Z[:], 0.0)
        nrmp = keep.tile([128, 8], f32, name="nrmp")
        cdp = keep.tile([128, 8], f32, name="cdp")
        # seg squares on DVE in fp16 (2x ports)
        scrA = keep.tile([128, 4, FD], f16, name="scrA")
        nc.vector.tensor_tensor(out=scrA[:], in0=big0[:, :, 0:FD],
                                in1=big0[:, :, 0:FD], op=Alu.mult)
        nc.vector.reduce_sum(out=nrmp[:, 0:4], in_=scrA[:], axis=Ax.X)
        scrB = keep.tile([128, 4, FD], f16, name="scrB")
        nc.vector.tensor_tensor(out=scrB[:], in0=big1[:, :, 0:FD],
                                in1=big1[:, :, 0:FD], op=Alu.mult)
        red_b = nc.vector.reduce_sum(out=nrmp[:, 4:8], in_=scrB[:], axis=Ax.X)
        nc.vector.tensor_scalar_max(out=cdp[:, 0:4], in0=big0[:, :, FD],
                                    scalar1=1.0)
        nc.vector.tensor_scalar_max(out=cdp[:, 4:8], in0=big1[:, :, FD],
                                    scalar1=1.0)
        rcdp = keep.tile([128, 8], f32, name="rcdp")
        nc.vector.reciprocal(rcdp[:], cdp[:])
        termp = keep.tile([128, 8], f32, name="termp")
        nc.vector.tensor_tensor(out=termp[:], in0=nrmp[:], in1=rcdp[:],
                                op=Alu.mult)
        nc.vector.reduce_sum(out=Z[:, 0:1], in_=termp[:], axis=Ax.X)

        scrU = keep.tile([64, FD], f32, name="scrU")
        uvtt = nc.vector.tensor_tensor(out=scrU[:], in0=ut[:, 0:FD],
                                       in1=vt[:, 0:FD], op=Alu.mult)
        # segment squares (gated only by big0/big1) must run before the
        # ut/vt-gated ops, or the whole DVE chain waits on the slower queue
        add_dep_helper(uvtt.ins, red_b.ins, sync=False,
                       reason="squares before stats ops")
        nc.vector.reduce_sum(out=Z[0:64, 1:2], in_=scrU[:], axis=Ax.X)
        nc.vector.tensor_tensor(out=Z[0:64, 2:3], in0=vt[:, FD + 1:FD + 2],
                                in1=ut[:, FD:FD + 1], op=Alu.mult)     # Psq*L
        nc.vector.tensor_tensor(out=Z[0:64, 3:4], in0=vt[:, FD:FD + 1],
                                in1=ut[:, FD + 1:FD + 2], op=Alu.mult)  # Pbar*Lsq
        scrF = keep.tile([1, FD], f32, name="scrF")
        nc.vector.tensor_tensor(out=scrF[:], in0=last2[:, 0:FD],
                                in1=r1[:, 0:FD], op=Alu.mult)
        nc.vector.reduce_sum(out=Z[0:1, 4:5], in_=scrF[:], axis=Ax.X)  # Fe.F

        zred = psum.tile([1, 8], f32, name="zred", tag="ps_0")
        nc.tensor.matmul(out=zred[:], lhsT=ones128[:], rhs=Z[:],
                         start=True, stop=True)
        zs = keep.tile([1, 8], f32, name="zs")
        nc.vector.tensor_copy(out=zs[:], in_=zred[:])

        # scalars: M=last2[256], a=last2[257], e=r1[256], se=r1[257]
        Mvh = last2[0:1, FD:FD + 1]
        avh = last2[0:1, FD + 1:FD + 2]
        evh = r1[0:1, FD:FD + 1]
        sevh = r1[0:1, FD + 1:FD + 2]
        sc32 = keep.tile([1, 4], f32, name="sc32")
        nc.vector.tensor_copy(out=sc32[0:1, 0:1], in_=Mvh)
        nc.vector.tensor_copy(out=sc32[0:1, 1:2], in_=avh)
        nc.vector.tensor_copy(out=sc32[0:1, 2:3], in_=evh)
        nc.vector.tensor_copy(out=sc32[0:1, 3:4], in_=sevh)
        Mv = sc32[0:1, 0:1]
        av = sc32[0:1, 1:2]
        ev = sc32[0:1, 2:3]
        sev = sc32[0:1, 3:4]
        s_center = zs[0:1, 0:1]
        uv = zs[0:1, 1:2]
        psql = zs[0:1, 2:3]
        pbarlsq = zs[0:1, 3:4]
        fef = zs[0:1, 4:5]

        fin = keep.tile([1, 16], f32, name="fin")
        nc.vector.memset(fin[:], 0.0)
        t_ = lambda i: fin[0:1, i:i + 1]
        # f0 = se*M ; f1 = a*e ; f2 = f0+f1
        nc.vector.tensor_tensor(out=t_(8), in0=sev, in1=Mv, op=Alu.mult)
        nc.vector.tensor_tensor(out=t_(9), in0=av, in1=ev, op=Alu.mult)
        nc.vector.tensor_tensor(out=t_(10), in0=t_(8), in1=t_(9), op=Alu.add)
        # f3 = -2*fef + f2
        nc.vector.tensor_scalar(out=t_(11), in0=fef, scalar1=-2.0,
                                scalar2=t_(10), op0=Alu.mult, op1=Alu.add)
        # f4 = f3 - psql ; f5 = f4 - pbarlsq
        nc.vector.tensor_tensor(out=t_(12), in0=t_(11), in1=psql, op=Alu.subtract)
        nc.vector.tensor_tensor(out=t_(13), in0=t_(12), in1=pbarlsq, op=Alu.subtract)
        # SD = 2*uv + f5
        nc.vector.tensor_scalar(out=t_(14), in0=uv, scalar1=2.0,
                                scalar2=t_(13), op0=Alu.mult, op1=Alu.add)
        # md = M*(M-1) ; rmd = 1/md ; div = SD*rmd*(-1/D)
        nc.vector.tensor_scalar(out=t_(15), in0=Mv, scalar1=-1.0,
                                scalar2=Mv, op0=Alu.add, op1=Alu.mult)
        nc.vector.reciprocal(t_(15), t_(15))
        nc.vector.tensor_tensor(out=t_(1), in0=t_(14), in1=t_(15), op=Alu.mult)
        nc.vector.tensor_scalar_mul(out=t_(1), in0=t_(1), scalar1=-1.0 / D)
        # tight = (a - s_center)/M
        nc.vector.tensor_tensor(out=t_(7), in0=av, in1=s_center, op=Alu.subtract)
        nc.vector.reciprocal(t_(6), Mv)
        nc.vector.tensor_tensor(out=t_(2), in0=t_(7), in1=t_(6), op=Alu.mult)
        # total = 0.1*div + 0.1*tight
        nc.vector.tensor_tensor(out=t_(0), in0=t_(1), in1=t_(2), op=Alu.add)
        nc.vector.tensor_scalar_mul(out=t_(0), in0=t_(0), scalar1=0.1)

        nc.sync.dma_start(out=outd[None, :], in_=fin[0:1, 0:8])

    nc.finalize()
    return nc


def _get_compiled(all_ones=True):
    key = "ones" if all_ones else "mask"
    if key not in _compiled:
        _compiled[key] = _build_bass(all_ones)
    return _compiled[key]


def _make_in_maps(features, targets, mask, all_ones=True):
    features = np.ascontiguousarray(np.asarray(features, dtype=np.float32))
    targets = np.ascontiguousarray(np.asarray(targets, dtype=np.float32))
    maskf = np.asarray(mask).astype(np.float32).reshape(B, 1)
    in_maps = []
    for i in range(NCORES):
        sl = slice(i * RB, (i + 1) * RB)
        im = {
            "features": features[sl],
            "targets": targets[sl],
        }
        if not all_ones:
            im["maskf"] = np.ascontiguousarray(maskf[sl])
        in_maps.append(im)
    return in_maps


def kernel(features, targets, mask):
    from concourse.bass_utils import run_bass_kernel_spmd

    all_ones = bool(np.all(np.asarray(mask)))
    nc = _get_compiled(all_ones)
    in_maps = _make_in_maps(features, targets, mask, all_ones)
    res = run_bass_kernel_spmd(nc, in_maps, list(range(NCORES)))
    out = res.results[0]["out"]
    total = np.float32(out[0])
    diversity = np.float32(out[1])
    tightness = np.float32(out[2])
    return total, diversity, tightness


# revision 40
# speedup vs baseline: 1.2814x; 1.2814x over previous
"""Trainium2 Bass kernel for CategoricalEntropyRegLoss.

Math: both loss terms factor so the [B,B] pairwise matrices are never built.

  feat_dists = sq_j + sq_k - 2 fn_j.fn_k            (rank FD+2)
  target_dists = (E_j - P_j.LQ_k) / D               (rank DC+1)
  S = sum_{jk} m_j m_k feat_dists * target_dists    (diag is exactly 0)
    = [ se*M + a*e - 2 Fe.F - Psq.L - Pbar.Lsq + 2 <U,V> ] / D
  tightness*M = a - sum_s ||seg_sum_s||^2 / max(cnt_s,1)

Everything needed is one matmul per core:
  out[1154, 258] = ext_seg^T @ ext_feat
  ext_seg  = [ onehot(code) | LQ' | P | 1 | E' ]    (B x 1154)
  ext_feat = [ m*fn | m | m ]                       (B x 258)
followed by a single 8-core AllReduce of the [128,10,258] partials and
a cheap redundant epilogue on every core.

Precision: all matmul operands and the AllReduce payload are fp16.
Raw log-probs (~ -3.5) would make the E/LQ-derived scalars (e, se ~
-1.7e3 per core) lose ~8 ulps per fp16 ring hop, which the ~7x
cancellation in the total amplifies past the threshold. The pairwise
KL is invariant under LQ += a, E += D*a, so we use LQ' = Ln(32*p)
(adds ln 32) and compute E' from LQ' -- this centers every stats value
near zero, the epilogue is unchanged, and end-to-end rel err lands at
~2.4e-4 measured vs the 2e-2 gate.

Scheduling notes (from perfetto traces):
 - fp32 matmul runs at 1/4 PE rate; fp16 operands cut the matmul phase
   ~20us -> ~5us. PSUM accumulates fp32 either way.
 - the collective costs ~11us fixed setup after its trigger plus the
   mesh phases (~19us for the 660KB fp16 payload); splitting into two
   collectives serializes on the one CC core and pays the fixed costs
   twice -- a single fused payload wins.
 - argmax is invariant under the positive per-row prob normalization,
   so the one-hot chain runs on raw targets the moment they land.
 - minv = m/||x|| comes from one ACT Abs_reciprocal_sqrt op (all-ones
   mask fast path; measured no accuracy impact) so the ef scale never
   waits on the DVE queue. A general masked variant (Sqrt + DVE
   reciprocal) is compiled on demand when mask has zeros.
 - inputs load with row-interleaved layout (partition p holds rows
   4p..4p+3): every reduced quantity is row-permutation invariant and
   descriptor lines go from 1KB/256B to 4KB/1KB contiguous.
 - cross-core launch skew (0-20us, run-to-run) is absorbed at the
   collective rendezvous; it is harness noise, not kernel time.
"""

import numpy as np

B = 4096
FD = 256
C = 32
D = 2
NSEG = C ** D          # 1024
NCORES = 8
RB = B // NCORES       # 512 rows per core
KT = RB // 128         # 4 k-chunks of 128 rows
EF = FD + 2            # 258: [mfn | m | m]
NST = 2 * D * C + 2    # 130 stats columns: [lq | p | ones | E]
NSMT = NSEG // 128     # 8 seg m-tiles
NMT = NSMT + 2         # 10 payload m-tiles

_compiled = {}


def _build_bass(all_ones):
    from contextlib import ExitStack
    import concourse.bass as bass
    import concourse.bacc as bacc
    import concourse.tile as tile
    from concourse import mybir

    from concourse.tile import add_dep_helper

    f32 = mybir.dt.float32
    f16 = mybir.dt.float16
    Alu = mybir.AluOpType
    Act = mybir.ActivationFunctionType
    Ax = mybir.AxisListType

    nc = bacc.Bacc(num_devices=NCORES)

    feat = nc.dram_tensor("features", [RB, FD], f32, kind="ExternalInput")
    targ = nc.dram_tensor("targets", [RB, D * C], f32, kind="ExternalInput")
    maskf = None
    if not all_ones:
        maskf = nc.dram_tensor("maskf", [RB, 1], f32, kind="ExternalInput")
    outd = nc.dram_tensor("out", [8], f32, kind="ExternalOutput")

    with ExitStack() as ctx:
        tc = ctx.enter_context(tile.TileContext(nc))
        consts = ctx.enter_context(tc.tile_pool(name="consts", bufs=1))
        work = ctx.enter_context(tc.tile_pool(name="work", bufs=1))
        keep = ctx.enter_context(tc.tile_pool(name="keep", bufs=1))
        psum = ctx.enter_context(tc.tile_pool(name="psum", bufs=1, space="PSUM"))
        dram = ctx.enter_context(tc.tile_pool(name="dram", bufs=1, space="DRAM"))

        # ---------------- constants ----------------
        ones128 = consts.tile([128, 1], f32)
        nc.vector.memset(ones128[:], 1.0)

        # partition-major fp16 payload [p, mt, f]: mt 0-7 seg, 8-9 stats
        inb = dram.tile([128, NMT, EF], f16, name="inb")
        outb = dram.tile([128, NMT, EF], f16, name="outb", addr_space="Shared")

        # ---- input loads spread over the queues (chunk-wise for early
        # arrival; squares are input-gated) ----
        # row-interleaved layout: partition p holds rows 4p..4p+3, chunk a =
        # a-th row of each partition. Every reduced quantity is permutation
        # invariant over rows, and this layout gives 4KB/1KB contiguous
        # descriptor lines per partition instead of 1KB/256B.
        tbig = keep.tile([128, KT, D * C], f32, name="tbig")
        nc.scalar.dma_start(
            out=tbig[:], in_=targ[:, :].rearrange("(p a) f -> p a f", p=128))
        mkbig = None
        if not all_ones:
            mkbig = keep.tile([128, KT, 1], f32, name="mkbig")
            nc.scalar.dma_start(
                out=mkbig[:],
                in_=maskf[:, :].rearrange("(p a) f -> p a f", p=128))
        xbig = keep.tile([128, KT, FD], f32, name="xbig")
        xflat = xbig[:, :, :].rearrange("p a f -> p (a f)")
        nc.sync.dma_start(
            out=xflat[0:44, :],
            in_=feat[0:176, :].rearrange("(p a) f -> p (a f)", p=44))
        nc.gpsimd.dma_start(
            out=xflat[44:88, :],
            in_=feat[176:352, :].rearrange("(p a) f -> p (a f)", p=44))
        nc.scalar.dma_start(
            out=xflat[88:128, :],
            in_=feat[352:512, :].rearrange("(p a) f -> p (a f)", p=40))

        def xchunk(kc):
            return xbig[:, kc, :]

        # iota after the gpsimd input DMA triggers (fp16: ints exact to 2048)
        iota1024 = consts.tile([128, NSEG], f16)
        nc.gpsimd.iota(iota1024[:], [[1, NSEG]], channel_multiplier=0,
                       allow_small_or_imprecise_dtypes=True)
        # biota[j] = 32 - j  (for first-argmax via reduce_max)
        biota = consts.tile([128, C], f32)
        nc.gpsimd.iota(biota[:], [[-1, C]], base=C, channel_multiplier=0,
                       allow_small_or_imprecise_dtypes=True)

        es_oh = [keep.tile([128, NSEG], f16, name=f"esoh_{kc}")
                 for kc in range(KT)]
        es_st = [keep.tile([128, NST], f32, name=f"esst_{kc}")
                 for kc in range(KT)]
        es_stb = [keep.tile([128, NST], f16, name=f"esstb_{kc}")
                  for kc in range(KT)]
        ef_b = [keep.tile([128, EF], f16, name=f"efb_{kc}")
                for kc in range(KT)]
        for kc in range(KT):
            nc.vector.memset(es_st[kc][:, NST - 2:NST - 1], 1.0)

        # ---- ACT phase 1: row sum-of-squares (Square table loads once) ----
        sqpack = keep.tile([128, KT], f32, name="sqpack")
        scrsq = keep.tile([128, FD], f32, name="scrsq")
        act_chain = []
        for kc in range(KT):
            act_chain.append(nc.scalar.activation(
                out=scrsq[:], in_=xchunk(kc), func=Act.Square,
                accum_out=sqpack[:, kc:kc + 1]))

        # ---- targets chains (DVE) ----
        # es_st columns: [0:64 lq' | 64:128 p | 128 ones | 129 E']
        t1big = keep.tile([128, KT, D * C], f32, name="t1big")
        nc.vector.tensor_scalar_add(out=t1big[:], in0=tbig[:], scalar1=1e-10)
        invsb = keep.tile([128, KT * D], f32, name="invsb")
        nc.vector.reduce_sum(
            out=invsb[:],
            in_=t1big[:].rearrange("p a (d c) -> p (a d) c", c=C),
            axis=Ax.X)
        nc.vector.reciprocal(invsb[:], invsb[:])
        # per-chunk first-argmax + one-hot on RAW targets (argmax is
        # invariant under the positive per-row normalization), DVE only —
        # runs the moment targets land, independent of the norm chain
        def argmax_onehot(kc):
            cls = work.tile([128, D], f32, name=f"cls_{kc}", tag=f"cl_{kc}")
            for d_ in range(D):
                pch = t1big[:, kc, C * d_:C * (d_ + 1)]
                mx = work.tile([128, 1], f32, name=f"mx_{kc}_{d_}",
                               tag=f"mx_{kc}_{d_}")
                nc.vector.reduce_max(out=mx[:], in_=pch, axis=Ax.X)
                cand = work.tile([128, C], f32, name=f"cand_{kc}_{d_}",
                                 tag=f"cd_{kc}_{d_}")
                # (p == max) * (32 - idx); reduce_max -> 32 - first_argmax
                nc.vector.scalar_tensor_tensor(
                    out=cand[:], in0=pch, scalar=mx[:], in1=biota[:],
                    op0=Alu.is_equal, op1=Alu.mult)
                mq = work.tile([128, 1], f32, name=f"mq_{kc}_{d_}",
                               tag=f"mq_{kc}_{d_}")
                nc.vector.reduce_max(out=mq[:], in_=cand[:], axis=Ax.X)
                nc.vector.tensor_scalar(
                    out=cls[:, d_:d_ + 1], in0=mq[:], scalar1=-1.0,
                    scalar2=float(C), op0=Alu.mult, op1=Alu.add)
            code = work.tile([128, 1], f32, name=f"code_{kc}", tag=f"co_{kc}")
            nc.vector.tensor_scalar(
                out=code[:], in0=cls[:, 1:2], scalar1=float(C),
                scalar2=cls[:, 0:1], op0=Alu.mult, op1=Alu.add)
            # one-hot straight to fp16 (ints <= 1023 exact)
            nc.vector.tensor_scalar(
                out=es_oh[kc][:], in0=iota1024[:], scalar1=code[:],
                scalar2=None, op0=Alu.is_equal)

        for kc in range(KT):
            argmax_onehot(kc)

        # ---- minv = m/norm, kept off the DVE critical queue where possible
        minvpack = keep.tile([128, KT], f32, name="minvpack")
        if all_ones:
            # single ACT op right after the squares
            act_chain.append(nc.scalar.activation(
                out=minvpack[:], in_=sqpack[:],
                func=Act.Abs_reciprocal_sqrt))
        else:
            normpack = keep.tile([128, KT], f32, name="normpack")
            act_chain.append(nc.scalar.sqrt(normpack[:], sqpack[:]))
            nc.vector.tensor_scalar_max(out=normpack[:], in0=normpack[:],
                                        scalar1=1e-12)
            invpack = keep.tile([128, KT], f32, name="invpack")
            nc.vector.reciprocal(invpack[:], normpack[:])
            nc.vector.tensor_tensor(out=minvpack[:], in0=invpack[:],
                                    in1=mkbig[:, :, 0], op=Alu.mult)

        # ---- ext_feat = [x*(m*inv) | m | m] in fp16 (ACT phase 3,
        # Copy is table-less so ef is ready before the Lns) ----
        for kc in range(KT):
            ef_t = ef_b[kc]
            act_chain.append(nc.scalar.activation(
                out=ef_t[:, 0:FD], in_=xchunk(kc), func=Act.Copy,
                scale=minvpack[:, kc:kc + 1]))
            if all_ones:
                nc.vector.memset(ef_t[:, FD:FD + 2], 1.0)
            else:
                nc.vector.tensor_copy(out=ef_t[:, FD:FD + 1],
                                      in_=mkbig[:, kc, :])
                nc.vector.tensor_copy(out=ef_t[:, FD + 1:FD + 2],
                                      in_=mkbig[:, kc, :])

        # ---- normalized probs (needed only by the stats operands) ----
        for kc in range(KT):
            pt = es_st[kc][:, D * C:2 * D * C]
            for d_ in range(D):
                nc.vector.tensor_scalar_mul(
                    out=pt[:, C * d_:C * (d_ + 1)],
                    in0=t1big[:, kc, C * d_:C * (d_ + 1)],
                    scalar1=invsb[:, kc * D + d_:kc * D + d_ + 1])

        # ---- ACT phase 4: Lns (stats operands are needed last) ----
        # lq' = Ln(32*p) = Ln(p) + ln 32: centers the log-probs so the
        # fp16 payload ring stays well conditioned (KL is invariant)
        for kc in range(KT):
            act_chain.append(nc.scalar.activation(
                out=es_st[kc][:, 0:D * C], in_=es_st[kc][:, D * C:2 * D * C],
                func=Act.Ln, scale=32.0))

        # E' / ones columns + fp16 operand copy of the stats block
        for kc in range(KT):
            st_t = es_st[kc]
            scr64 = work.tile([128, D * C], f32, name=f"scr64_{kc}",
                              tag=f"s64_{kc}")
            nc.vector.tensor_tensor(out=scr64[:],
                                    in0=st_t[:, D * C:2 * D * C],
                                    in1=st_t[:, 0:D * C], op=Alu.mult)
            nc.vector.reduce_sum(out=st_t[:, NST - 1:NST], in_=scr64[:],
                                 axis=Ax.X)
            nc.vector.tensor_copy(out=es_stb[kc][:], in_=st_t[:])

        # keep ACT ops grouped by function (avoid act-table reload thrash):
        # Square x4 -> Ln x4 -> Sqrt -> Copy x4 (table-less)
        for a, b in zip(act_chain[1:], act_chain[:-1]):
            add_dep_helper(a.ins, b.ins, sync=False,
                           reason="act table grouping")

        # ---------------- the one big matmul ----------------
        resa = keep.tile([128, 4, EF], f16, name="resa")
        resb = keep.tile([128, 4, EF], f16, name="resb")
        resc = keep.tile([128, 2, EF], f16, name="resc")
        nc.vector.memset(resc[:], 0.0)
        for mt in range(NMT):
            mlo = mt * 128
            msz = min(128, 1154 - mlo)
            ps = psum.tile([msz, EF], f32, name=f"ps_{mt}", tag=f"ps_{mt % 7}")
            for kc in range(KT):
                if mt < NSMT:
                    lhsT = es_oh[kc][:, mlo:mlo + msz]
                else:
                    lhsT = es_stb[kc][:, mlo - NSEG:mlo - NSEG + msz]
                nc.tensor.matmul(out=ps[:], lhsT=lhsT, rhs=ef_b[kc][:],
                                 start=(kc == 0), stop=(kc == KT - 1))
            if mt < 4:
                nc.vector.tensor_copy(out=resa[:, mt, :], in_=ps[:])
            elif mt < 8:
                nc.vector.tensor_copy(out=resb[:, mt - 4, :], in_=ps[:])
            else:
                nc.vector.tensor_copy(out=resc[0:msz, mt - 8, :], in_=ps[:])
            if mt == 3:
                nc.sync.dma_start(out=inb[:, 0:4, :], in_=resa[:])
            elif mt == 7:
                nc.gpsimd.dma_start(out=inb[:, 4:8, :], in_=resb[:])
            elif mt == 9:
                nc.scalar.dma_start(out=inb[:, 8:10, :], in_=resc[:])

        # ---------------- single AllReduce (fp16) ----------------
        nc.gpsimd.collective_compute(
            "AllReduce", mybir.AluOpType.add,
            replica_groups=[list(range(NCORES))],
            ins=[inb.opt()], outs=[outb.opt()])

        # ---------------- epilogue (redundant on every core) ----------------
        big0 = keep.tile([128, 4, EF], f16, name="big0")
        nc.sync.dma_start(out=big0[:], in_=outb[:, 0:4, :])
        big1 = keep.tile([128, 4, EF], f16, name="big1")
        nc.gpsimd.dma_start(out=big1[:], in_=outb[:, 4:8, :])
        # stats rows: partition-shifted copies so DVE lanes align
        ut = keep.tile([64, EF], f16, name="ut")
        nc.scalar.dma_start(out=ut[:], in_=outb[0:64, 8, :])
        vt = keep.tile([64, EF], f16, name="vt")
        nc.scalar.dma_start(out=vt[:], in_=outb[64:128, 8, :])
        last2 = keep.tile([1, EF], f16, name="last2")
        nc.scalar.dma_start(out=last2[:], in_=outb[0:1, 9, :])
        r1 = keep.tile([1, EF], f16, name="r1")
        nc.sync.dma_start(out=r1[:], in_=outb[1:2, 9, :])

        Z = keep.tile([128, 8], f32, name="Z")
        nc.vector.memset(Z[:], 0.0)
        Zc = keep.tile([128, 1], f32, name="Zc")
        nrmp = keep.tile([128, 8], f32, name="nrmp")
        cdp = keep.tile([128, 8], f32, name="cdp")

        # scalars: M=last2[256], a=last2[257], e=r1[256], se=r1[257];
        # stage to f32 and precompute the reciprocals right after the small
        # loads land, off the tail critical path
        sc32 = keep.tile([1, 8], f32, name="sc32")
        nc.vector.tensor_copy(out=sc32[0:1, 0:2], in_=last2[0:1, FD:FD + 2])
        nc.vector.tensor_copy(out=sc32[0:1, 2:4], in_=r1[0:1, FD:FD + 2])
        Mv = sc32[0:1, 0:1]
        av = sc32[0:1, 1:2]
        ev = sc32[0:1, 2:3]
        sev = sc32[0:1, 3:4]
        rmd = sc32[0:1, 4:5]   # 1/(M*(M-1))
        rM = sc32[0:1, 5:6]    # 1/M
        nc.vector.tensor_scalar(out=rmd, in0=Mv, scalar1=-1.0,
                                scalar2=Mv, op0=Alu.add, op1=Alu.mult)
        nc.vector.reciprocal(rmd, rmd)
        nc.vector.reciprocal(rM, Mv)
        # seg squares on DVE in fp16 (2x ports)
        scrA = keep.tile([128, 4, FD], f16, name="scrA")
        nc.vector.tensor_tensor(out=scrA[:], in0=big0[:, :, 0:FD],
                                in1=big0[:, :, 0:FD], op=Alu.mult)
        nc.vector.reduce_sum(out=nrmp[:, 0:4], in_=scrA[:], axis=Ax.X)
        scrB = keep.tile([128, 4, FD], f16, name="scrB")
        nc.vector.tensor_tensor(out=scrB[:], in0=big1[:, :, 0:FD],
                                in1=big1[:, :, 0:FD], op=Alu.mult)
        red_b = nc.vector.reduce_sum(out=nrmp[:, 4:8], in_=scrB[:], axis=Ax.X)
        nc.vector.tensor_scalar_max(out=cdp[:, 0:4], in0=big0[:, :, FD],
                                    scalar1=1.0)
        nc.vector.tensor_scalar_max(out=cdp[:, 4:8], in0=big1[:, :, FD],
                                    scalar1=1.0)
        rcdp = keep.tile([128, 8], f32, name="rcdp")
        nc.vector.reciprocal(rcdp[:], cdp[:])
        termp = keep.tile([128, 8], f32, name="termp")
        nc.vector.tensor_tensor(out=termp[:], in0=nrmp[:], in1=rcdp[:],
                                op=Alu.mult)
        nc.vector.reduce_sum(out=Zc[:], in_=termp[:], axis=Ax.X)

        scrU = keep.tile([64, FD], f32, name="scrU")
        uvtt = nc.vector.tensor_tensor(out=scrU[:], in0=ut[:, 0:FD],
                                       in1=vt[:, 0:FD], op=Alu.mult)
        # segment squares (gated only by big0/big1) must run before the
        # ut/vt-gated ops, or the whole DVE chain waits on the slower queue
        add_dep_helper(uvtt.ins, red_b.ins, sync=False,
                       reason="squares before stats ops")
        nc.vector.reduce_sum(out=Z[0:64, 1:2], in_=scrU[:], axis=Ax.X)
        nc.vector.tensor_tensor(out=Z[0:64, 2:3], in0=vt[:, FD + 1:FD + 2],
                                in1=ut[:, FD:FD + 1], op=Alu.mult)     # Psq*L
        nc.vector.tensor_tensor(out=Z[0:64, 3:4], in0=vt[:, FD:FD + 1],
                                in1=ut[:, FD + 1:FD + 2], op=Alu.mult)  # Pbar*Lsq
        scrF = keep.tile([1, FD], f32, name="scrF")
        nc.vector.tensor_tensor(out=scrF[:], in0=last2[:, 0:FD],
                                in1=r1[:, 0:FD], op=Alu.mult)
        nc.vector.reduce_sum(out=Z[0:1, 4:5], in_=scrF[:], axis=Ax.X)  # Fe.F

        # split Z reduction: the stats columns complete ~2us before the
        # s_center column (gated by the big nrmp reduces), so the long
        # diversity leg of the scalar chain runs in that shadow
        zred = psum.tile([1, 8], f32, name="zred", tag="ps_0")
        nc.tensor.matmul(out=zred[:], lhsT=ones128[:], rhs=Z[:],
                         start=True, stop=True)
        zs = keep.tile([1, 8], f32, name="zs")
        nc.vector.tensor_copy(out=zs[:], in_=zred[:])
        uv = zs[0:1, 1:2]
        psql = zs[0:1, 2:3]
        pbarlsq = zs[0:1, 3:4]
        fef = zs[0:1, 4:5]

        fin = keep.tile([1, 16], f32, name="fin")
        nc.vector.memset(fin[:], 0.0)
        t_ = lambda i: fin[0:1, i:i + 1]
        # f0 = se*M ; f1 = a*e ; f2 = f0+f1
        nc.vector.tensor_tensor(out=t_(8), in0=sev, in1=Mv, op=Alu.mult)
        nc.vector.tensor_tensor(out=t_(9), in0=av, in1=ev, op=Alu.mult)
        nc.vector.tensor_tensor(out=t_(10), in0=t_(8), in1=t_(9), op=Alu.add)
        # f3 = -2*fef + f2
        nc.vector.tensor_scalar(out=t_(11), in0=fef, scalar1=-2.0,
                                scalar2=t_(10), op0=Alu.mult, op1=Alu.add)
        # f4 = f3 - psql ; f5 = f4 - pbarlsq
        nc.vector.tensor_tensor(out=t_(12), in0=t_(11), in1=psql, op=Alu.subtract)
        nc.vector.tensor_tensor(out=t_(13), in0=t_(12), in1=pbarlsq, op=Alu.subtract)
        # SD = 2*uv + f5 ; div = SD*rmd*(-1/D)
        nc.vector.tensor_scalar(out=t_(14), in0=uv, scalar1=2.0,
                                scalar2=t_(13), op0=Alu.mult, op1=Alu.add)
        nc.vector.tensor_tensor(out=t_(1), in0=t_(14), in1=rmd, op=Alu.mult)
        nc.vector.tensor_scalar_mul(out=t_(1), in0=t_(1), scalar1=-1.0 / D)

        # center leg: tiny [1,1] reduction of Zc, then 3 ops to the total
        zredc = psum.tile([1, 1], f32, name="zredc", tag="ps_1")
        nc.tensor.matmul(out=zredc[:], lhsT=ones128[:], rhs=Zc[:],
                         start=True, stop=True)
        zsc = keep.tile([1, 1], f32, name="zsc")
        nc.vector.tensor_copy(out=zsc[:], in_=zredc[:])
        # tight = (a - s_center)/M
        nc.vector.tensor_tensor(out=t_(7), in0=av, in1=zsc[0:1, 0:1],
                                op=Alu.subtract)
        nc.vector.tensor_tensor(out=t_(2), in0=t_(7), in1=rM, op=Alu.mult)
        # total = 0.1*div + 0.1*tight
        nc.vector.tensor_tensor(out=t_(0), in0=t_(1), in1=t_(2), op=Alu.add)
        nc.vector.tensor_scalar_mul(out=t_(0), in0=t_(0), scalar1=0.1)

        nc.sync.dma_start(out=outd[None, :], in_=fin[0:1, 0:8])

    nc.finalize()
    return nc


def _get_compiled(all_ones=True):
    key = "ones" if all_ones else "mask"
    if key not in _compiled:
        _compiled[key] = _build_bass(all_ones)
    return _compiled[key]


def _make_in_maps(features, targets, mask, all_ones=True):
    features = np.ascontiguousarray(np.asarray(features, dtype=np.float32))
    targets = np.ascontiguousarray(np.asarray(targets, dtype=np.float32))
    maskf = np.asarray(mask).astype(np.float32).reshape(B, 1)
    in_maps = []
    for i in range(NCORES):
        sl = slice(i * RB, (i + 1) * RB)
        im = {
            "features": features[sl],
            "targets": targets[sl],
        }
        if not all_ones:
            im["maskf"] = np.ascontiguousarray(maskf[sl])
        in_maps.append(im)
    return in_maps


def kernel(features, targets, mask):
    from concourse.bass_utils import run_bass_kernel_spmd

    all_ones = bool(np.all(np.asarray(mask)))
    nc = _get_compiled(all_ones)
    in_maps = _make_in_maps(features, targets, mask, all_ones)
    res = run_bass_kernel_spmd(nc, in_maps, list(range(NCORES)))
    out = res.results[0]["out"]
    total = np.float32(out[0])
    diversity = np.float32(out[1])
    tightness = np.float32(out[2])
    return total, diversity, tightness
